# revision 2
# baseline (speedup 1.0000x reference)
"""3D Haar wavelet transform (2x2x2, causal temporal pad) on 8 Trainium2 cores.

Input  x: (2, 3, 33, 512, 512) fp32
Output y: (2, 24, 17, 256, 256) fp32   (channel = 3*s + c, s = subband)

Sharding: pure data parallel over H — core ci handles input rows
[64*ci, 64*ci+64) i.e. output rows [32*ci, 32*ci+32).

The host pre-gathers each core's input into layout x'[b, c, i, h, T', w]
(i = temporal offset in the 2-frame pair, T' = output frame; causal pad
baked in by clamping), so every DMA is fully contiguous on both sides.

Per-core pipeline, per (b, c):
  1 in-DMA  -> A[128, 17*512]  (partition p = i*64 + h, free = (T', w);
               4.45 MB, fully contiguous 34KB/partition rows, SP ring)
  per T': DVE sum/diff of adjacent w pairs (W-axis Haar stage)
          -> B[128, (sw, w')] in float32r
          one 128x128 matmul (T+H Haar stages + partition permutation,
          fixed +-1 weights; out partition m = di*64 + dj*32 + q)
          one ACT mul evacuates PSUM -> C[128, (T', sw, w')] with x0.3536
  1 out-DMA -> y'[b, c, m, T', sw, w']  (4.45 MB contiguous, ACT ring)
Host reorders y' -> y (subband-major channels, h' concat).

DMA coalescing is the point: 12 x 4.45MB transfers amortize the ~2us
per-dma_start fixed cost; per-core HBM traffic is 53.4MB -> ~149us floor
at 358 GB/s.
"""

import numpy as np

import concourse.bacc as bacc
import concourse.mybir as mybir
from concourse import tile
from concourse.bass_utils import run_bass_kernel_spmd

P = 128
B_, C_, T_, H_, W_ = 2, 3, 33, 512, 512
NCORES = 8
HC = H_ // NCORES          # 64 input rows per core
TP = (T_ + 1) // 2         # 17 output frames
HP = HC // 2               # 32 output rows per core
WP = W_ // 2               # 256 output cols
SCALE = float(np.float32(0.3536))
F32 = mybir.dt.float32
# float32r: TF32-like single-pass matmul (4x faster PE than fp32's 2-pass,
# ~1e-4 rel err). Set False for bit-accurate fp32 (2-pass, slower).
USE_FP32R = True
FREE = TP * W_             # 8704 elements per partition per (b, c)


def _haar_matrix() -> np.ndarray:
    """W[p, m]: p = i*64 + h (h = 2q+j), m = di*64 + dj*32 + q, val (-1)^(i*di+j*dj)."""
    W = np.zeros((P, P), dtype=np.float32)
    for i in range(2):
        for h in range(HC):
            j = h & 1
            q = h >> 1
            for di in range(2):
                for dj in range(2):
                    m = di * 64 + dj * 32 + q
                    W[i * 64 + h, m] = (-1.0) ** (i * di + j * dj)
    return W


def build_nc():
    nc = bacc.Bacc("TRN2", target_bir_lowering=False, debug=False)
    # x': [b, c, i, h, T', w] host-pretransposed, pad baked in
    x_d = nc.dram_tensor("x", [B_, C_, 2, HC, TP, W_], F32, kind="ExternalInput")
    # y': [b, c, m, T', sw, w']  (m = output partition; frame-major free dim
    # so the whole (b, c) out-tile is one contiguous DMA)
    y_d = nc.dram_tensor("y", [B_, C_, P, TP, 2, WP], F32, kind="ExternalOutput")
    w_d = nc.inline_tensor(_haar_matrix(), name="haar_w")

    mm_dt = mybir.dt.float32r if USE_FP32R else F32

    with tile.TileContext(nc) as tc:
        with (
            tc.tile_pool(name="wpool", bufs=1) as wpool,
            tc.tile_pool(name="apool", bufs=3) as apool,
            tc.tile_pool(name="bpool", bufs=4) as bpool,
            tc.tile_pool(name="cpool", bufs=2) as cpool,
            tc.tile_pool(name="psum", bufs=8, space="PSUM") as psum_pool,
        ):
            w_sb = wpool.tile([P, P], mm_dt)
            (nc.gpsimd if USE_FP32R else nc.sync).dma_start(
                out=w_sb[:], in_=w_d[:]
            )

            for b in range(B_):
                for c in range(C_):
                    xin = x_d[b, c].rearrange("i h T w -> (i h) (T w)")
                    yout = y_d[b, c].rearrange("m T s w -> m (T s w)")
                    # one fully-contiguous 4.45MB load on the SP HWDGE ring
                    a = apool.tile([P, FREE], F32, tag="a")
                    nc.sync.dma_start(out=a[:], in_=xin[:])
                    cbig = cpool.tile([P, FREE], F32, tag="c")
                    for tg in range(TP):
                        # W-axis stage: sum/diff of adjacent w pairs
                        av = a[:, tg * W_ : (tg + 1) * W_].rearrange(
                            "p (w k) -> p k w", k=2
                        )
                        bt = bpool.tile([P, W_], mm_dt)
                        nc.vector.tensor_add(
                            out=bt[:, 0:WP], in0=av[:, 0], in1=av[:, 1]
                        )
                        nc.vector.tensor_sub(
                            out=bt[:, WP:W_], in0=av[:, 0], in1=av[:, 1]
                        )
                        # T+H stages as one matmul
                        ps = psum_pool.tile([P, W_], F32)
                        nc.tensor.matmul(
                            ps[:], w_sb[:], bt[:], start=True, stop=True
                        )
                        # evacuate + scale into the (T', sw, w') frame slot
                        nc.scalar.mul(
                            cbig[:, tg * W_ : (tg + 1) * W_], ps[:], SCALE
                        )
                    # one fully-contiguous 4.45MB store on the ACT HWDGE ring
                    nc.scalar.dma_start(out=yout[:], in_=cbig[:])
    nc.compile()
    return nc


_NC_CACHE = None


def _get_nc():
    global _NC_CACHE
    if _NC_CACHE is None:
        _NC_CACHE = build_nc()
    return _NC_CACHE


# xp[tp] = x[max(tp-1, 0)] (causal pad); pair (T', i) reads xp[2T'+i]
_TIDX = np.maximum(np.arange(2 * TP) - 1, 0)


def _prep_core_input(x: np.ndarray, ci: int) -> np.ndarray:
    xc = x[:, :, _TIDX, HC * ci : HC * (ci + 1), :]      # [2,3,34,64,512]
    xc = xc.reshape(B_, C_, TP, 2, HC, W_)               # [b,c,T',i,h,w]
    return np.ascontiguousarray(xc.transpose(0, 1, 3, 4, 2, 5))  # [b,c,i,h,T',w]


def kernel(x: np.ndarray) -> np.ndarray:
    assert x.shape == (B_, C_, T_, H_, W_), x.shape
    x = np.ascontiguousarray(x, dtype=np.float32)
    nc = _get_nc()
    in_maps = [{"x": _prep_core_input(x, ci)} for ci in range(NCORES)]
    res = run_bass_kernel_spmd(nc, in_maps, core_ids=list(range(NCORES)))
    y = np.empty((B_, 8 * C_, TP, H_ // 2, WP), dtype=np.float32)
    for ci in range(NCORES):
        yc = res.results[ci]["y"]                        # [b,c,128,17,2,256]
        yc = yc.reshape(B_, C_, 4, HP, TP, 2, WP)        # m = m4*32+q
        yc = yc.transpose(0, 2, 5, 1, 4, 3, 6)           # [b,m4,sw,c,T',q,w']
        yc = yc.reshape(B_, 8 * C_, TP, HP, WP)          # ch = (2*m4+sw)*3+c
        y[:, :, :, HP * ci : HP * (ci + 1), :] = yc
    return y


# revision 3
# speedup vs baseline: 1.9519x; 1.9519x over previous
"""3D Haar wavelet transform (2x2x2, causal temporal pad) on 8 Trainium2 cores.

Input  x: (2, 3, 33, 512, 512) fp32
Output y: (2, 24, 17, 256, 256) fp32   (channel = 3*s + c, s = subband)

Sharding: pure data parallel over H — core ci handles input rows
[64*ci, 64*ci+64) i.e. output rows [32*ci, 32*ci+32).

All three Haar stages (T, H, W) fold into ONE 128x128 matmul by putting
the three 2x2x2-block parities on the partition axis:
  input partition  p = i*64 + j*32 + k*16 + qlo
    (i = temporal offset, j = h parity, k = w parity, qlo = q mod 16
     where h = 2q + j, w = 2w' + k, q = qhi*16 + qlo)
  output partition m = di*64 + dj*32 + dw*16 + qlo, subband s = 4di+2dj+dw
  W[p, m] = (-1)^(i*di + j*dj + k*dw) iff qlo matches (8 nonzeros/col)
Free dim carries (qhi, T', w') = 2*17*256 = 8704 values per partition.

Everything runs in bf16 (the 2e-2 rel-err budget dwarfs bf16's ~2e-3):
host casts input, kernel writes bf16, host upcasts the gathered output.
That halves HBM traffic to 13.4MB in + 13.4MB out per core.

Per-core pipeline, per (b, c):
  1 in-DMA  [128, 8704] bf16 (2.23MB contiguous, SP HWDGE ring)
  17x matmul [128p, 512-chunk] -> PSUM fp32
  17x evacuate PSUM -> C bf16 with x0.3536, alternating ACT / DVE
  1 out-DMA [128, 8704] bf16 (2.23MB contiguous, ACT HWDGE ring)
Host reorders y' -> y (subband-major channels, h' concat) and upcasts.
"""

import numpy as np
import ml_dtypes

import concourse.bacc as bacc
import concourse.mybir as mybir
from concourse import tile
from concourse.bass_utils import run_bass_kernel_spmd

P = 128
B_, C_, T_, H_, W_ = 2, 3, 33, 512, 512
NCORES = 8
HC = H_ // NCORES          # 64 input rows per core
TP = (T_ + 1) // 2         # 17 output frames
HP = HC // 2               # 32 output rows per core
WP = W_ // 2               # 256 output cols
SCALE = float(np.float32(0.3536))
F32 = mybir.dt.float32
BF16 = mybir.dt.bfloat16
BF16_NP = ml_dtypes.bfloat16
FREE = 2 * TP * WP         # 8704 = (qhi, T', w') per partition per (b, c)
NCHUNK = FREE // 512       # 17 matmul chunks of 512


def _haar_matrix() -> np.ndarray:
    """W[p, m] with p = i*64+j*32+k*16+qlo, m = di*64+dj*32+dw*16+qlo."""
    W = np.zeros((P, P), dtype=np.float32)
    for i in range(2):
        for j in range(2):
            for k in range(2):
                for q in range(16):
                    p = i * 64 + j * 32 + k * 16 + q
                    for di in range(2):
                        for dj in range(2):
                            for dw in range(2):
                                m = di * 64 + dj * 32 + dw * 16 + q
                                W[p, m] = (-1.0) ** (i * di + j * dj + k * dw)
    return W.astype(BF16_NP)


def build_nc():
    nc = bacc.Bacc("TRN2", target_bir_lowering=False, debug=False)
    # x': [b, c, i, j, k, qlo, qhi, T', w'] host-pretransposed bf16
    x_d = nc.dram_tensor(
        "x", [B_, C_, 2, 2, 2, 16, 2, TP, WP], BF16, kind="ExternalInput"
    )
    # y': [b, c, m, qhi, T', w'] bf16
    y_d = nc.dram_tensor("y", [B_, C_, P, 2, TP, WP], BF16, kind="ExternalOutput")
    w_d = nc.inline_tensor(_haar_matrix(), name="haar_w")

    with tile.TileContext(nc) as tc:
        with (
            tc.tile_pool(name="wpool", bufs=1) as wpool,
            tc.tile_pool(name="apool", bufs=3) as apool,
            tc.tile_pool(name="cpool", bufs=3) as cpool,
            tc.tile_pool(name="psum", bufs=8, space="PSUM") as psum_pool,
        ):
            w_sb = wpool.tile([P, P], BF16)
            nc.sync.dma_start(out=w_sb[:], in_=w_d[:])

            for b in range(B_):
                for c in range(C_):
                    xin = x_d[b, c].rearrange("i j k q Q T w -> (i j k q) (Q T w)")
                    yout = y_d[b, c].rearrange("m Q T w -> m (Q T w)")
                    a = apool.tile([P, FREE], BF16, tag="a")
                    nc.sync.dma_start(out=a[:], in_=xin[:])
                    cbig = cpool.tile([P, FREE], BF16, tag="c")
                    for tg in range(NCHUNK):
                        sl = slice(tg * 512, (tg + 1) * 512)
                        ps = psum_pool.tile([P, 512], F32)
                        nc.tensor.matmul(
                            ps[:], w_sb[:], a[:, sl], start=True, stop=True
                        )
                        # evacuate + scale; alternate engines so neither
                        # ACT nor DVE exceeds the DMA budget
                        if tg % 2 == 0:
                            nc.scalar.mul(cbig[:, sl], ps[:], SCALE)
                        else:
                            nc.vector.tensor_scalar_mul(cbig[:, sl], ps[:], SCALE)
                    nc.scalar.dma_start(out=yout[:], in_=cbig[:])
    nc.compile()
    return nc


_NC_CACHE = None


def _get_nc():
    global _NC_CACHE
    if _NC_CACHE is None:
        _NC_CACHE = build_nc()
    return _NC_CACHE


# xp[tp] = x[max(tp-1, 0)] (causal pad); pair (T', i) reads xp[2T'+i]
_TIDX = np.maximum(np.arange(2 * TP) - 1, 0)


def _prep_core_input(xbf: np.ndarray, ci: int) -> np.ndarray:
    xc = xbf[:, :, _TIDX, HC * ci : HC * (ci + 1), :]    # [2,3,34,64,512] bf16
    # [b,c,T',i,(q,j)->h,(w',k)->w] split h and w into (quotient, parity)
    xc = xc.reshape(B_, C_, TP, 2, 2, 16, 2, WP, 2)      # [b,c,T',i,qh,ql,j,w',k]
    xc = xc.transpose(0, 1, 3, 6, 8, 5, 4, 2, 7)         # [b,c,i,j,k,ql,qh,T',w']
    return np.ascontiguousarray(xc)


def kernel(x: np.ndarray) -> np.ndarray:
    assert x.shape == (B_, C_, T_, H_, W_), x.shape
    xbf = np.asarray(x, dtype=np.float32).astype(BF16_NP)
    nc = _get_nc()
    in_maps = [{"x": _prep_core_input(xbf, ci)} for ci in range(NCORES)]
    res = run_bass_kernel_spmd(nc, in_maps, core_ids=list(range(NCORES)))
    y = np.empty((B_, 8 * C_, TP, H_ // 2, WP), dtype=np.float32)
    for ci in range(NCORES):
        yc = np.asarray(res.results[ci]["y"])            # [b,c,128,2,17,256] bf16
        yc = yc.reshape(B_, C_, 2, 2, 2, 16, 2, TP, WP)  # [b,c,di,dj,dw,ql,qh,T,w']
        yc = yc.transpose(0, 2, 3, 4, 1, 7, 6, 5, 8)     # [b,di,dj,dw,c,T,qh,ql,w']
        yc = yc.reshape(B_, 8 * C_, TP, HP, WP)          # ch = (4di+2dj+dw)*3+c
        y[:, :, :, HP * ci : HP * (ci + 1), :] = yc.astype(np.float32)
    return y
